# revision 9
# baseline (speedup 1.0000x reference)
"""Trainium2 Bass kernel for nn_MemoryReader (sparse_attention), v2.

Reference computation (per batch b):
  s[m,q]  = sum_c K_M[b,c,m] * K_Q[b,c,q] / sqrt(64)        m in [0,9216), q in [0,2304)
  attn    = softmax over m
  mem[c,q]= sum_m V_M[b,c,m] * attn[m,q]                    c in [0,128)
  E       = concat([mem, V_Q[b]], ch)                       [256, q]
  out     = relu(bn_scale * (conv_w @ E) + bn_shift)        [64, q]

Sharding: 8 cores = (B=4) x (Q halves of 1152). Fully data-parallel.

v2 design (vs v1 two-pass bf16 pipeline):
  - q processed in 3 passes of 512/512/128 columns; every matmul output is
    exactly one PSUM bank (zero-region aligned). PSUM: mem accumulator
    (1 bank) + softmax-denominator d (1 bank) + 3 score slots (2 banks each).
  - K and V quantized to fp8e4m3 on host; QK and (most) PV matmuls run in
    DoubleRow perf mode (2 stacked k-tiles, 0.5 cycles/col).
  - The exp is split across two engines to break the ACT bottleneck:
    ACT pairs -> activation Exp with scale=1/8 (the 1/sqrt(C_K)) and
    bias=ln(1/4) (softmax-invariant 2^-2 range shift for fp8), output
    fp8e4m3 consumed by DoubleRow PV; per-pair denominator partial goes to
    PSUM d via a DoubleRow ones-matmul on PE.
    DVE pairs -> Schraudolph exp: i16 = trunc(s*(log2e*128/8) + B) written
    through an int16 bitcast of a bf16 tile gives 2^-2*exp(s/8) to ~2% rms
    (softmax-tolerant); consumed by plain bf16 PV; denominator partials
    accumulate on DVE in bf16 g, folded into d by one bf16 ones-matmul.
  - Epilogue per pass: r = 1/d (DVE), mem_r = mem * r -> bf16 (DVE),
    y = w1 @ mem_r + w2 @ vq in one PSUM group (PE), out = Relu(y + shift)
    on ACT with per-partition bias.
"""

import numpy as np
import ml_dtypes

import concourse.bass as bass
from concourse import bacc
import concourse.mybir as mybir
import concourse.tile as tile

B, C_K, C_V, NN, H, W = 4, 64, 128, 4, 48, 48
M = NN * H * W          # 9216
Q = H * W               # 2304
QH = Q // 2             # 1152 per core
OUT_CH = 64
BN_EPS = 1e-5
NCORES = 8
MT = M // 128           # 72 m-tiles, 36 pairs
NPAIR = MT // 2
BF16 = mybir.dt.bfloat16
F32 = mybir.dt.float32
FP8 = mybir.dt.float8e4
FP8E5 = mybir.dt.float8e5
I8 = mybir.dt.int8
AF = mybir.ActivationFunctionType
DR = mybir.MatmulPerfMode.DoubleRow

# Schraudolph exp constants (fp8e5m2 bitcast via int8: 2^-3 range shift,
# 1/8 score scale; i8 = trunc(s*A + B), safe/positive for |s/8| < 8)
EXP_A = 1.4426950408889634 * 4.0 / 8.0       # 0.7213475
EXP_B = 48.25
# ACT path: exp(s/8 + ln(0.125)) = 2^-3 * exp(s/8)
ACT_SCALE = 0.125
ACT_BIAS = float(np.log(0.125))

PASSES = [(0, 512), (512, 512), (1024, 128)]


_DVE_SET_0 = frozenset(g for g in range(NPAIR) if g % 2 == 1 and g != 17)
_DVE_SET_1 = _DVE_SET_0


def pair_is_dve(g, qs=0):
    """Pair assignment for the 512-passes: 17 (pass 0) / 16 (pass 1)."""
    return g in (_DVE_SET_0 if qs == 0 else _DVE_SET_1)


def quad_is_dve(qd):
    """Quad assignment for the 128-pass: 9 DVE quads of 18."""
    return qd % 2 == 1 and qd != 17




def _emit(nc, aps, reps=1):
    km8, kq8, vt8, vq, w12, shift, out = aps
    with tile.TileContext(nc) as tc:
        with (
            tc.tile_pool(name="consts", bufs=1) as consts,
            tc.tile_pool(name="p8p", bufs=8) as p8p,
            tc.tile_pool(name="ep", bufs=2) as ep,
            tc.tile_pool(name="obuf", bufs=1) as obuf,
            tc.tile_pool(name="spool", bufs=3, space="PSUM") as spool,
            tc.tile_pool(name="mpool", bufs=1, space="PSUM") as mpool,
            tc.tile_pool(name="dpool", bufs=1, space="PSUM") as dpool,
        ):
            km8_t = consts.tile([32, MT, 2, 128], FP8)
            kq8_t = consts.tile([32, 2, QH], FP8)
            vt8_t = consts.tile([128, NPAIR, 2, 128], FP8)
            vq_t = consts.tile([128, QH], BF16)
            w12_t = consts.tile([128, 2 * OUT_CH], BF16)
            shift_t = consts.tile([OUT_CH, 1], F32)
            ones8_t = consts.tile([128, 2, 128], FP8)
            expb_t = consts.tile([128, 1], F32)
            w1_t = w12_t[:, 0:OUT_CH]
            w2_t = w12_t[:, OUT_CH:2 * OUT_CH]

            # input DMAs, chunked so early tiles unblock first
            nc.sync.dma_start(out=kq8_t[:, :, 0:512], in_=kq8[:, :, 0:512])
            nc.sync.dma_start(out=km8_t[:, 0:2], in_=km8[:, 0:2])
            nc.sync.dma_start(out=vt8_t[:, 0:2], in_=vt8[:, 0:2])
            nc.sync.dma_start(out=km8_t[:, 2:12], in_=km8[:, 2:12])
            nc.sync.dma_start(out=vt8_t[:, 2:12], in_=vt8[:, 2:12])
            nc.sync.dma_start(out=km8_t[:, 12:36], in_=km8[:, 12:36])
            nc.sync.dma_start(out=vt8_t[:, 12:NPAIR], in_=vt8[:, 12:NPAIR])
            nc.sync.dma_start(out=km8_t[:, 36:MT], in_=km8[:, 36:MT])
            nc.sync.dma_start(out=kq8_t[:, :, 512:QH], in_=kq8[:, :, 512:QH])
            nc.sync.dma_start(out=vq_t, in_=vq)
            nc.sync.dma_start(out=w12_t, in_=w12)
            nc.sync.dma_start(out=shift_t, in_=shift)
            nc.vector.memset(ones8_t, 1.0)
            nc.vector.memset(expb_t, ACT_BIAS)

            o_t = obuf.tile([OUT_CH, QH], F32, tag="o")

            # epilogue of the previous pass, deferred into the next pass's
            # prologue so next-pass QKs are not stuck behind it in the
            # in-order PE queue
            epi_prev = [None]

            for rep in range(reps):
              for (qs, qw) in PASSES:
                mem_t = mpool.tile([128, 512], F32, tag="mem")
                d_t = dpool.tile([128, 512], F32, tag="d")
                # software pipeline: QK+exp run LAG pair-groups ahead of the
                # PE-side PV/d matmuls and the DVE denominator adds, so the
                # in-order PE queue never blocks on an exp
                LAG = 5 if qw == 512 else 9
                deferred = []   # list of closures, one per pair-group

                def flush_one():
                    for fn in deferred.pop(0):
                        fn()

                if qw == 512:
                    # 36 pairs of m-tiles; exp on ACT (fp8e4m3) or DVE
                    # (Schraudolph -> fp8e5m2); identical DoubleRow PV and
                    # ones-matmul denominator downstream
                    for g in range(NPAIR):
                        s_t = spool.tile([128, 2, 512], F32, tag="s")
                        for i in (0, 1):
                            nc.tensor.matmul(
                                s_t[:, i, 0:qw], km8_t[:, 2 * g + i],
                                kq8_t[:, :, qs:qs + qw],
                                start=True, stop=True, perf_mode=DR)
                        if not pair_is_dve(g, qs):
                            p_t = p8p.tile([128, 2, 512], FP8, tag="p8")
                            nc.scalar.activation(
                                out=p_t[:, :, 0:qw], in_=s_t[:, :, 0:qw],
                                func=AF.Exp, bias=expb_t[:, 0:1],
                                scale=ACT_SCALE)
                        else:
                            p_t = p8p.tile([128, 2, 512], FP8E5, tag="p5")
                            nc.vector.tensor_scalar(
                                out=p_t[:, :, 0:qw].bitcast(I8),
                                in0=s_t[:, :, 0:qw],
                                scalar1=EXP_A, scalar2=EXP_B,
                                op0=mybir.AluOpType.mult,
                                op1=mybir.AluOpType.add)

                        def pv_ops(g=g, p_t=p_t):
                            nc.tensor.matmul(
                                mem_t[:, 0:qw], vt8_t[:, g], p_t[:, :, 0:qw],
                                start=(g == 0), stop=(g == NPAIR - 1),
                                perf_mode=DR)
                            nc.tensor.matmul(
                                d_t[:, 0:qw], ones8_t, p_t[:, :, 0:qw],
                                start=(g == 0), stop=(g == NPAIR - 1),
                                perf_mode=DR)
                        deferred.append([pv_ops])
                        if len(deferred) > LAG:
                            flush_one()
                        if g == 9 and epi_prev[0] is not None:
                            epi_prev[0]()
                            epi_prev[0] = None
                else:
                    # last pass, qw=128: quads of 4 m-tiles
                    for qd in range(MT // 4):
                        s_t = spool.tile([128, 4, 128], F32, tag="s")
                        for i in range(4):
                            nc.tensor.matmul(
                                s_t[:, i], km8_t[:, 4 * qd + i],
                                kq8_t[:, :, qs:qs + qw],
                                start=True, stop=True, perf_mode=DR)
                        if not quad_is_dve(qd):
                            p_t = p8p.tile([128, 4, 128], FP8, tag="p8")
                            nc.scalar.activation(
                                out=p_t, in_=s_t, func=AF.Exp,
                                bias=expb_t[:, 0:1], scale=ACT_SCALE)
                        else:
                            p_t = p8p.tile([128, 4, 128], FP8E5, tag="p5")
                            nc.vector.tensor_scalar(
                                out=p_t.bitcast(I8), in0=s_t,
                                scalar1=EXP_A, scalar2=EXP_B,
                                op0=mybir.AluOpType.mult,
                                op1=mybir.AluOpType.add)

                        def pv_ops(qd=qd, p_t=p_t):
                            for j in range(2):
                                gg = 2 * qd + j
                                nc.tensor.matmul(
                                    mem_t[:, 0:qw], vt8_t[:, gg],
                                    p_t[:, 2 * j:2 * j + 2],
                                    start=(gg == 0), stop=(gg == NPAIR - 1),
                                    perf_mode=DR)
                                nc.tensor.matmul(
                                    d_t[:, 0:qw], ones8_t,
                                    p_t[:, 2 * j:2 * j + 2],
                                    start=(gg == 0), stop=(gg == NPAIR - 1),
                                    perf_mode=DR)
                        deferred.append([pv_ops])
                        if len(deferred) > LAG:
                            flush_one()
                        if qd == 6 and epi_prev[0] is not None:
                            epi_prev[0]()
                            epi_prev[0] = None
                while deferred:
                    flush_one()

                # r and mem_r can run now (DVE is idle at pass end), the
                # PE/ACT/SP parts of the epilogue are deferred
                r_t = ep.tile([128, 512], F32, tag="r")
                last = (rep == reps - 1 and qw == 128)
                nc.vector.reciprocal(out=r_t[:, 0:qw], in_=d_t[:, 0:qw])
                memr_t = ep.tile([128, 512], BF16, tag="memr")
                nc.vector.tensor_mul(memr_t[:, 0:qw], mem_t[:, 0:qw],
                                     r_t[:, 0:qw])

                def epi(qs=qs, qw=qw, memr_t=memr_t, last=last):
                    y_t = spool.tile([64, 512], F32, tag="s")
                    nc.tensor.matmul(y_t[:, 0:qw], w1_t, memr_t[:, 0:qw],
                                     start=True, stop=False)
                    nc.tensor.matmul(y_t[:, 0:qw], w2_t, vq_t[:, qs:qs + qw],
                                     start=False, stop=True)
                    nc.scalar.activation(out=o_t[:, qs:qs + qw],
                                         in_=y_t[:, 0:qw],
                                         func=AF.Relu, bias=shift_t[:, 0:1])
                    nc.sync.dma_start(out=out[:, qs:qs + qw],
                                      in_=o_t[:, qs:qs + qw])
                epi_prev[0] = epi
            if epi_prev[0] is not None:
                epi_prev[0]()
                epi_prev[0] = None


def _build_nc(reps=1):
    nc = bacc.Bacc("TRN2", target_bir_lowering=False, debug=False)
    km8 = nc.dram_tensor("km8", [32, MT, 2, 128], FP8, kind="ExternalInput").ap()
    kq8 = nc.dram_tensor("kq8", [32, 2, QH], FP8, kind="ExternalInput").ap()
    vt8 = nc.dram_tensor("vt8", [128, NPAIR, 2, 128], FP8,
                         kind="ExternalInput").ap()
    vq = nc.dram_tensor("vq", [128, QH], BF16, kind="ExternalInput").ap()
    w12 = nc.dram_tensor("w12", [128, 2 * OUT_CH], BF16,
                         kind="ExternalInput").ap()
    shift = nc.dram_tensor("shift", [OUT_CH, 1], F32, kind="ExternalInput").ap()
    out = nc.dram_tensor("out", [OUT_CH, QH], F32, kind="ExternalOutput").ap()
    _emit(nc, (km8, kq8, vt8, vq, w12, shift, out), reps=reps)
    nc.compile()
    return nc


def prepare_in_maps(K_M, V_M, K_Q, V_Q, conv_w, bn_gamma, bn_beta, bn_mean, bn_var):
    """Host-side shard + layout prep. Returns list of 8 per-core input dicts."""
    bf16 = ml_dtypes.bfloat16
    fp8 = ml_dtypes.float8_e4m3
    K_M = np.asarray(K_M, np.float32)
    V_M = np.asarray(V_M, np.float32)
    K_Q = np.asarray(K_Q, np.float32)
    V_Q = np.asarray(V_Q, np.float32)
    conv_w = np.asarray(conv_w, np.float32)
    scale = np.asarray(bn_gamma, np.float32) / np.sqrt(
        np.asarray(bn_var, np.float32) + BN_EPS)
    shift = (np.asarray(bn_beta, np.float32)
             - np.asarray(bn_mean, np.float32) * scale)
    w_eff = conv_w * scale[:, None]
    w1t = np.ascontiguousarray(w_eff[:, :C_V].T).astype(bf16)   # [128, 64]
    w2t = np.ascontiguousarray(w_eff[:, C_V:].T).astype(bf16)   # [128, 64]

    in_maps = []
    for b in range(B):
        km = K_M[b].reshape(C_K, MT, 128)                # [64, 72, 128]
        # km8[p, t, j, m] = km[j*32+p, t, m]
        km8 = np.ascontiguousarray(
            km.reshape(2, 32, MT, 128).transpose(1, 2, 0, 3)).astype(fp8)

        v = V_M[b].reshape(C_V, NPAIR, 2, 128)           # [128, 36, 2, 128]
        # vt8[m, g, i, c] = v[c, g, i, m]
        vt8 = np.ascontiguousarray(v.transpose(3, 1, 2, 0)).astype(fp8)

        kq = K_Q[b].reshape(C_K, Q)
        vq_full = V_Q[b].reshape(C_V, Q)
        w12 = np.concatenate([w1t, w2t], axis=1)         # [128, 128] bf16
        for h in range(2):
            sl = slice(h * QH, (h + 1) * QH)
            kq8 = np.ascontiguousarray(
                kq[:, sl].reshape(2, 32, QH).transpose(1, 0, 2)).astype(fp8)
            in_maps.append({
                "km8": km8,
                "kq8": kq8,
                "vt8": vt8,
                "vq": vq_full[:, sl].astype(bf16),
                "w12": w12,
                "shift": shift.reshape(OUT_CH, 1).astype(np.float32),
            })
    return in_maps


def assemble_output(results):
    """results: list of 8 dicts with 'out' [64, 1152] -> [4, 64, 48, 48] f32."""
    out = np.empty((B, OUT_CH, Q), np.float32)
    for c in range(NCORES):
        b, h = c // 2, c % 2
        out[b, :, h * QH:(h + 1) * QH] = results[c]["out"]
    return out.reshape(B, OUT_CH, H, W)


_RUNNERS = {}


def _get_runner(reps=1):
    """Build the Bass module + a cached sharded jit callable (compile once)."""
    if reps in _RUNNERS:
        return _RUNNERS[reps]
    import jax
    from jax.sharding import Mesh, PartitionSpec
    from jax.experimental.shard_map import shard_map
    from concourse import bass2jax

    nc = _build_nc(reps=reps)
    bass2jax.install_neuronx_cc_hook()

    partition_name = nc.partition_id_tensor.name if nc.partition_id_tensor else None
    in_names, out_names, out_avals, zero_outs = [], [], [], []
    for alloc in nc.m.functions[0].allocations:
        if not isinstance(alloc, mybir.MemoryLocationSet):
            continue
        name = alloc.memorylocations[0].name
        if alloc.kind == "ExternalInput":
            if name != partition_name:
                in_names.append(name)
        elif alloc.kind == "ExternalOutput":
            out_names.append(name)
            shape = tuple(alloc.tensor_shape)
            dtype = mybir.dt.np(alloc.dtype)
            out_avals.append(jax.core.ShapedArray(shape, dtype))
            zero_outs.append(np.zeros(shape, dtype))
    n_params = len(in_names)
    n_outs = len(out_avals)
    all_in_names = in_names + out_names
    if partition_name is not None:
        all_in_names = all_in_names + [partition_name]

    def _body(*args):
        operands = list(args)
        if partition_name is not None:
            operands.append(bass2jax.partition_id_tensor())
        outs = bass2jax._bass_exec_p.bind(
            *operands,
            out_avals=tuple(out_avals),
            in_names=tuple(all_in_names),
            out_names=tuple(out_names),
            lowering_input_output_aliases=(),
            sim_require_finite=True,
            sim_require_nnan=True,
            nc=nc,
        )
        return tuple(outs)

    devices = jax.devices()[:NCORES]
    assert len(devices) == NCORES, f"need {NCORES} devices, got {len(jax.devices())}"
    mesh = Mesh(np.asarray(devices), ("core",))
    in_specs = (PartitionSpec("core"),) * (n_params + n_outs)
    out_specs = (PartitionSpec("core"),) * n_outs
    donate = tuple(range(n_params, n_params + n_outs))
    sharded = jax.jit(
        shard_map(_body, mesh=mesh, in_specs=in_specs, out_specs=out_specs,
                  check_rep=False),
        donate_argnums=donate, keep_unused=True,
    )
    _RUNNERS[reps] = (sharded, in_names, out_names, out_avals, zero_outs)
    return _RUNNERS[reps]


def run_cores(in_maps):
    """Run the 8-core SPMD program; returns per-core output dicts."""
    sharded, in_names, out_names, out_avals, zero_outs = _get_runner()
    concat_in = [
        np.concatenate([np.asarray(in_maps[c][n]) for c in range(NCORES)], axis=0)
        for n in in_names
    ]
    concat_zeros = [
        np.zeros((NCORES * z.shape[0], *z.shape[1:]), z.dtype) for z in zero_outs
    ]
    out_arrs = sharded(*concat_in, *concat_zeros)
    return [
        {
            name: np.asarray(out_arrs[i]).reshape(NCORES, *out_avals[i].shape)[c]
            for i, name in enumerate(out_names)
        }
        for c in range(NCORES)
    ]


def kernel(K_M, V_M, K_Q, V_Q, conv_w, bn_gamma, bn_beta, bn_mean, bn_var):
    in_maps = prepare_in_maps(K_M, V_M, K_Q, V_Q, conv_w,
                              bn_gamma, bn_beta, bn_mean, bn_var)
    results = run_cores(in_maps)
    return assemble_output(results)


# revision 10
# speedup vs baseline: 1.3537x; 1.3537x over previous
"""Trainium2 Bass kernel for nn_MemoryReader (sparse_attention), v2.

Reference computation (per batch b):
  s[m,q]  = sum_c K_M[b,c,m] * K_Q[b,c,q] / sqrt(64)        m in [0,9216), q in [0,2304)
  attn    = softmax over m
  mem[c,q]= sum_m V_M[b,c,m] * attn[m,q]                    c in [0,128)
  E       = concat([mem, V_Q[b]], ch)                       [256, q]
  out     = relu(bn_scale * (conv_w @ E) + bn_shift)        [64, q]

Sharding: 8 cores = (B=4) x (Q halves of 1152). Fully data-parallel.

v2 design (vs v1 two-pass bf16 pipeline):
  - q processed in 3 passes of 512/512/128 columns; every matmul output is
    exactly one PSUM bank (zero-region aligned). PSUM: mem accumulator
    (1 bank) + softmax-denominator d (1 bank) + 3 score slots (2 banks each).
  - K and V quantized to fp8e4m3 on host; QK and (most) PV matmuls run in
    DoubleRow perf mode (2 stacked k-tiles, 0.5 cycles/col).
  - The exp is split across two engines to break the ACT bottleneck:
    ACT pairs -> activation Exp with scale=1/8 (the 1/sqrt(C_K)) and
    bias=ln(1/4) (softmax-invariant 2^-2 range shift for fp8), output
    fp8e4m3 consumed by DoubleRow PV; per-pair denominator partial goes to
    PSUM d via a DoubleRow ones-matmul on PE.
    DVE pairs -> Schraudolph exp: i16 = trunc(s*(log2e*128/8) + B) written
    through an int16 bitcast of a bf16 tile gives 2^-2*exp(s/8) to ~2% rms
    (softmax-tolerant); consumed by plain bf16 PV; denominator partials
    accumulate on DVE in bf16 g, folded into d by one bf16 ones-matmul.
  - Epilogue per pass: r = 1/d (DVE), mem_r = mem * r -> bf16 (DVE),
    y = w1 @ mem_r + w2 @ vq in one PSUM group (PE), out = Relu(y + shift)
    on ACT with per-partition bias.
"""

import numpy as np
import ml_dtypes

import concourse.bass as bass
from concourse import bacc
import concourse.mybir as mybir
import concourse.tile as tile

B, C_K, C_V, NN, H, W = 4, 64, 128, 4, 48, 48
M = NN * H * W          # 9216
Q = H * W               # 2304
QH = Q // 2             # 1152 per core
OUT_CH = 64
BN_EPS = 1e-5
NCORES = 8
MT = M // 128           # 72 m-tiles, 36 pairs
NPAIR = MT // 2
BF16 = mybir.dt.bfloat16
F32 = mybir.dt.float32
FP8 = mybir.dt.float8e4
FP8E5 = mybir.dt.float8e5
I8 = mybir.dt.int8
AF = mybir.ActivationFunctionType
DR = mybir.MatmulPerfMode.DoubleRow

# Schraudolph exp constants (fp8e5m2 bitcast via int8: 2^-3 range shift,
# 1/8 score scale; i8 = trunc(s*A + B), safe/positive for |s/8| < 8)
EXP_A = 1.4426950408889634 * 4.0 / 8.0       # 0.7213475
EXP_B = 48.25
# ACT path: exp(s/8 + ln(0.125)) = 2^-3 * exp(s/8)
ACT_SCALE = 0.125
ACT_BIAS = float(np.log(0.125))

PASSES = [(0, 512), (512, 512), (1024, 128)]


_DVE_SET_0 = frozenset(g for g in range(NPAIR) if g % 2 == 1 and g != 35)
_DVE_SET_1 = _DVE_SET_0


def pair_is_dve(g, qs=0):
    """Pair assignment for the 512-passes: 17 (pass 0) / 16 (pass 1)."""
    return g in (_DVE_SET_0 if qs == 0 else _DVE_SET_1)


def quad_is_dve(qd):
    """Quad assignment for the 128-pass: 9 DVE quads of 18."""
    return qd % 2 == 1 and qd != 17




def _emit(nc, aps, reps=1):
    km8, kq8, vt8, vq, w12, shift, out = aps
    with tile.TileContext(nc) as tc:
        with (
            tc.tile_pool(name="consts", bufs=1) as consts,
            tc.tile_pool(name="p8p", bufs=8) as p8p,
            tc.tile_pool(name="ep", bufs=2) as ep,
            tc.tile_pool(name="obuf", bufs=1) as obuf,
            tc.tile_pool(name="spool", bufs=3, space="PSUM") as spool,
            tc.tile_pool(name="mpool", bufs=1, space="PSUM") as mpool,
            tc.tile_pool(name="dpool", bufs=1, space="PSUM") as dpool,
        ):
            km8_t = consts.tile([32, MT, 2, 128], FP8)
            kq8_t = consts.tile([32, 2, QH], FP8)
            vt8_t = consts.tile([128, NPAIR, 2, 128], FP8)
            vq_t = consts.tile([128, QH], BF16)
            w12_t = consts.tile([128, 2 * OUT_CH], BF16)
            shift_t = consts.tile([OUT_CH, 1], F32)
            ones8_t = consts.tile([128, 2, 128], FP8)
            expb_t = consts.tile([128, 1], F32)
            w1_t = w12_t[:, 0:OUT_CH]
            w2_t = w12_t[:, OUT_CH:2 * OUT_CH]

            # input DMAs, chunked so early tiles unblock first
            nc.sync.dma_start(out=kq8_t[:, :, 0:512], in_=kq8[:, :, 0:512])
            nc.sync.dma_start(out=km8_t[:, 0:2], in_=km8[:, 0:2])
            nc.sync.dma_start(out=vt8_t[:, 0:2], in_=vt8[:, 0:2])
            nc.sync.dma_start(out=km8_t[:, 2:12], in_=km8[:, 2:12])
            nc.sync.dma_start(out=vt8_t[:, 2:12], in_=vt8[:, 2:12])
            nc.sync.dma_start(out=km8_t[:, 12:36], in_=km8[:, 12:36])
            nc.sync.dma_start(out=vt8_t[:, 12:NPAIR], in_=vt8[:, 12:NPAIR])
            nc.sync.dma_start(out=km8_t[:, 36:MT], in_=km8[:, 36:MT])
            nc.sync.dma_start(out=kq8_t[:, :, 512:QH], in_=kq8[:, :, 512:QH])
            nc.sync.dma_start(out=vq_t, in_=vq)
            nc.sync.dma_start(out=w12_t, in_=w12)
            nc.sync.dma_start(out=shift_t, in_=shift)
            nc.vector.memset(ones8_t, 1.0)
            nc.vector.memset(expb_t, ACT_BIAS)

            o_t = obuf.tile([OUT_CH, QH], F32, tag="o")

            # epilogue of the previous pass, deferred into the next pass's
            # prologue so next-pass QKs are not stuck behind it in the
            # in-order PE queue
            epi_prev = [None]

            for rep in range(reps):
              for (qs, qw) in PASSES:
                mem_t = mpool.tile([128, 512], F32, tag="mem")
                d_t = dpool.tile([128, 512], F32, tag="d")
                # software pipeline: QK+exp run LAG pair-groups ahead of the
                # PE-side PV/d matmuls and the DVE denominator adds, so the
                # in-order PE queue never blocks on an exp
                LAG = 5 if qw == 512 else 9
                deferred = []   # list of closures, one per pair-group

                def flush_one():
                    for fn in deferred.pop(0):
                        fn()

                if qw == 512:
                    # 36 pairs of m-tiles; exp on ACT (fp8e4m3) or DVE
                    # (Schraudolph -> fp8e5m2); identical DoubleRow PV and
                    # ones-matmul denominator downstream
                    for g in range(NPAIR):
                        s_t = spool.tile([128, 2, 512], F32, tag="s")
                        for i in (0, 1):
                            nc.tensor.matmul(
                                s_t[:, i, 0:qw], km8_t[:, 2 * g + i],
                                kq8_t[:, :, qs:qs + qw],
                                start=True, stop=True, perf_mode=DR)
                        if not pair_is_dve(g, qs):
                            p_t = p8p.tile([128, 2, 512], FP8, tag="p8")
                            nc.scalar.activation(
                                out=p_t[:, :, 0:qw], in_=s_t[:, :, 0:qw],
                                func=AF.Exp, bias=expb_t[:, 0:1],
                                scale=ACT_SCALE)
                        else:
                            p_t = p8p.tile([128, 2, 512], FP8E5, tag="p5")
                            nc.vector.tensor_scalar(
                                out=p_t[:, :, 0:qw].bitcast(I8),
                                in0=s_t[:, :, 0:qw],
                                scalar1=EXP_A, scalar2=EXP_B,
                                op0=mybir.AluOpType.mult,
                                op1=mybir.AluOpType.add)

                        def pv_ops(g=g, p_t=p_t):
                            nc.tensor.matmul(
                                mem_t[:, 0:qw], vt8_t[:, g], p_t[:, :, 0:qw],
                                start=(g == 0), stop=(g == NPAIR - 1),
                                perf_mode=DR)
                            nc.tensor.matmul(
                                d_t[:, 0:qw], ones8_t, p_t[:, :, 0:qw],
                                start=(g == 0), stop=(g == NPAIR - 1),
                                perf_mode=DR)
                        deferred.append([pv_ops])
                        if len(deferred) > LAG:
                            flush_one()
                        if g == 7 and epi_prev[0] is not None:
                            epi_prev[0]()
                            epi_prev[0] = None
                else:
                    # last pass, qw=128: quads of 4 m-tiles
                    for qd in range(MT // 4):
                        s_t = spool.tile([128, 4, 128], F32, tag="s")
                        for i in range(4):
                            nc.tensor.matmul(
                                s_t[:, i], km8_t[:, 4 * qd + i],
                                kq8_t[:, :, qs:qs + qw],
                                start=True, stop=True, perf_mode=DR)
                        if not quad_is_dve(qd):
                            p_t = p8p.tile([128, 4, 128], FP8, tag="p8")
                            nc.scalar.activation(
                                out=p_t, in_=s_t, func=AF.Exp,
                                bias=expb_t[:, 0:1], scale=ACT_SCALE)
                        else:
                            p_t = p8p.tile([128, 4, 128], FP8E5, tag="p5")
                            nc.vector.tensor_scalar(
                                out=p_t.bitcast(I8), in0=s_t,
                                scalar1=EXP_A, scalar2=EXP_B,
                                op0=mybir.AluOpType.mult,
                                op1=mybir.AluOpType.add)

                        def pv_ops(qd=qd, p_t=p_t):
                            for j in range(2):
                                gg = 2 * qd + j
                                nc.tensor.matmul(
                                    mem_t[:, 0:qw], vt8_t[:, gg],
                                    p_t[:, 2 * j:2 * j + 2],
                                    start=(gg == 0), stop=(gg == NPAIR - 1),
                                    perf_mode=DR)
                                nc.tensor.matmul(
                                    d_t[:, 0:qw], ones8_t,
                                    p_t[:, 2 * j:2 * j + 2],
                                    start=(gg == 0), stop=(gg == NPAIR - 1),
                                    perf_mode=DR)
                        deferred.append([pv_ops])
                        if len(deferred) > LAG:
                            flush_one()
                        if qd == 6 and epi_prev[0] is not None:
                            epi_prev[0]()
                            epi_prev[0] = None
                while deferred:
                    flush_one()

                # r and mem_r can run now (DVE is idle at pass end), the
                # PE/ACT/SP parts of the epilogue are deferred
                r_t = ep.tile([128, 512], F32, tag="r")
                last = (rep == reps - 1 and qw == 128)
                nc.vector.reciprocal(out=r_t[:, 0:qw], in_=d_t[:, 0:qw])
                memr_t = ep.tile([128, 512], BF16, tag="memr")
                nc.vector.tensor_mul(memr_t[:, 0:qw], mem_t[:, 0:qw],
                                     r_t[:, 0:qw])

                def epi(qs=qs, qw=qw, memr_t=memr_t, last=last):
                    y_t = spool.tile([64, 512], F32, tag="s")
                    nc.tensor.matmul(y_t[:, 0:qw], w1_t, memr_t[:, 0:qw],
                                     start=True, stop=False)
                    nc.tensor.matmul(y_t[:, 0:qw], w2_t, vq_t[:, qs:qs + qw],
                                     start=False, stop=True)
                    nc.scalar.activation(out=o_t[:, qs:qs + qw],
                                         in_=y_t[:, 0:qw],
                                         func=AF.Relu, bias=shift_t[:, 0:1])
                    nc.sync.dma_start(out=out[:, qs:qs + qw],
                                      in_=o_t[:, qs:qs + qw])
                epi_prev[0] = epi
            if epi_prev[0] is not None:
                epi_prev[0]()
                epi_prev[0] = None


def _build_nc(reps=1):
    nc = bacc.Bacc("TRN2", target_bir_lowering=False, debug=False)
    km8 = nc.dram_tensor("km8", [32, MT, 2, 128], FP8, kind="ExternalInput").ap()
    kq8 = nc.dram_tensor("kq8", [32, 2, QH], FP8, kind="ExternalInput").ap()
    vt8 = nc.dram_tensor("vt8", [128, NPAIR, 2, 128], FP8,
                         kind="ExternalInput").ap()
    vq = nc.dram_tensor("vq", [128, QH], BF16, kind="ExternalInput").ap()
    w12 = nc.dram_tensor("w12", [128, 2 * OUT_CH], BF16,
                         kind="ExternalInput").ap()
    shift = nc.dram_tensor("shift", [OUT_CH, 1], F32, kind="ExternalInput").ap()
    out = nc.dram_tensor("out", [OUT_CH, QH], F32, kind="ExternalOutput").ap()
    _emit(nc, (km8, kq8, vt8, vq, w12, shift, out), reps=reps)
    nc.compile()
    return nc


def prepare_in_maps(K_M, V_M, K_Q, V_Q, conv_w, bn_gamma, bn_beta, bn_mean, bn_var):
    """Host-side shard + layout prep. Returns list of 8 per-core input dicts."""
    bf16 = ml_dtypes.bfloat16
    fp8 = ml_dtypes.float8_e4m3
    K_M = np.asarray(K_M, np.float32)
    V_M = np.asarray(V_M, np.float32)
    K_Q = np.asarray(K_Q, np.float32)
    V_Q = np.asarray(V_Q, np.float32)
    conv_w = np.asarray(conv_w, np.float32)
    scale = np.asarray(bn_gamma, np.float32) / np.sqrt(
        np.asarray(bn_var, np.float32) + BN_EPS)
    shift = (np.asarray(bn_beta, np.float32)
             - np.asarray(bn_mean, np.float32) * scale)
    w_eff = conv_w * scale[:, None]
    w1t = np.ascontiguousarray(w_eff[:, :C_V].T).astype(bf16)   # [128, 64]
    w2t = np.ascontiguousarray(w_eff[:, C_V:].T).astype(bf16)   # [128, 64]

    in_maps = []
    for b in range(B):
        km = K_M[b].reshape(C_K, MT, 128)                # [64, 72, 128]
        # km8[p, t, j, m] = km[j*32+p, t, m]
        km8 = np.ascontiguousarray(
            km.reshape(2, 32, MT, 128).transpose(1, 2, 0, 3)).astype(fp8)

        v = V_M[b].reshape(C_V, NPAIR, 2, 128)           # [128, 36, 2, 128]
        # vt8[m, g, i, c] = v[c, g, i, m]
        vt8 = np.ascontiguousarray(v.transpose(3, 1, 2, 0)).astype(fp8)

        kq = K_Q[b].reshape(C_K, Q)
        vq_full = V_Q[b].reshape(C_V, Q)
        w12 = np.concatenate([w1t, w2t], axis=1)         # [128, 128] bf16
        for h in range(2):
            sl = slice(h * QH, (h + 1) * QH)
            kq8 = np.ascontiguousarray(
                kq[:, sl].reshape(2, 32, QH).transpose(1, 0, 2)).astype(fp8)
            in_maps.append({
                "km8": km8,
                "kq8": kq8,
                "vt8": vt8,
                "vq": vq_full[:, sl].astype(bf16),
                "w12": w12,
                "shift": shift.reshape(OUT_CH, 1).astype(np.float32),
            })
    return in_maps


def assemble_output(results):
    """results: list of 8 dicts with 'out' [64, 1152] -> [4, 64, 48, 48] f32."""
    out = np.empty((B, OUT_CH, Q), np.float32)
    for c in range(NCORES):
        b, h = c // 2, c % 2
        out[b, :, h * QH:(h + 1) * QH] = results[c]["out"]
    return out.reshape(B, OUT_CH, H, W)


_RUNNERS = {}


def _get_runner(reps=1):
    """Build the Bass module + a cached sharded jit callable (compile once)."""
    if reps in _RUNNERS:
        return _RUNNERS[reps]
    import jax
    from jax.sharding import Mesh, PartitionSpec
    from jax.experimental.shard_map import shard_map
    from concourse import bass2jax

    nc = _build_nc(reps=reps)
    bass2jax.install_neuronx_cc_hook()

    partition_name = nc.partition_id_tensor.name if nc.partition_id_tensor else None
    in_names, out_names, out_avals, zero_outs = [], [], [], []
    for alloc in nc.m.functions[0].allocations:
        if not isinstance(alloc, mybir.MemoryLocationSet):
            continue
        name = alloc.memorylocations[0].name
        if alloc.kind == "ExternalInput":
            if name != partition_name:
                in_names.append(name)
        elif alloc.kind == "ExternalOutput":
            out_names.append(name)
            shape = tuple(alloc.tensor_shape)
            dtype = mybir.dt.np(alloc.dtype)
            out_avals.append(jax.core.ShapedArray(shape, dtype))
            zero_outs.append(np.zeros(shape, dtype))
    n_params = len(in_names)
    n_outs = len(out_avals)
    all_in_names = in_names + out_names
    if partition_name is not None:
        all_in_names = all_in_names + [partition_name]

    def _body(*args):
        operands = list(args)
        if partition_name is not None:
            operands.append(bass2jax.partition_id_tensor())
        outs = bass2jax._bass_exec_p.bind(
            *operands,
            out_avals=tuple(out_avals),
            in_names=tuple(all_in_names),
            out_names=tuple(out_names),
            lowering_input_output_aliases=(),
            sim_require_finite=True,
            sim_require_nnan=True,
            nc=nc,
        )
        return tuple(outs)

    devices = jax.devices()[:NCORES]
    assert len(devices) == NCORES, f"need {NCORES} devices, got {len(jax.devices())}"
    mesh = Mesh(np.asarray(devices), ("core",))
    in_specs = (PartitionSpec("core"),) * (n_params + n_outs)
    out_specs = (PartitionSpec("core"),) * n_outs
    donate = tuple(range(n_params, n_params + n_outs))
    sharded = jax.jit(
        shard_map(_body, mesh=mesh, in_specs=in_specs, out_specs=out_specs,
                  check_rep=False),
        donate_argnums=donate, keep_unused=True,
    )
    _RUNNERS[reps] = (sharded, in_names, out_names, out_avals, zero_outs)
    return _RUNNERS[reps]


def run_cores(in_maps):
    """Run the 8-core SPMD program; returns per-core output dicts."""
    sharded, in_names, out_names, out_avals, zero_outs = _get_runner()
    concat_in = [
        np.concatenate([np.asarray(in_maps[c][n]) for c in range(NCORES)], axis=0)
        for n in in_names
    ]
    concat_zeros = [
        np.zeros((NCORES * z.shape[0], *z.shape[1:]), z.dtype) for z in zero_outs
    ]
    out_arrs = sharded(*concat_in, *concat_zeros)
    return [
        {
            name: np.asarray(out_arrs[i]).reshape(NCORES, *out_avals[i].shape)[c]
            for i, name in enumerate(out_names)
        }
        for c in range(NCORES)
    ]


def kernel(K_M, V_M, K_Q, V_Q, conv_w, bn_gamma, bn_beta, bn_mean, bn_var):
    in_maps = prepare_in_maps(K_M, V_M, K_Q, V_Q, conv_w,
                              bn_gamma, bn_beta, bn_mean, bn_var)
    results = run_cores(in_maps)
    return assemble_output(results)
